# revision 14
# baseline (speedup 1.0000x reference)
"""Trainium2 Bass kernel v5 for CWL2GCNLayer (WL2 GNN message passing).

reference:
    XW = X @ W; XW_prop = X @ W_prop; XW_back = X @ W_back
    S = relu(XW_prop[ref_a] + XW_prop[ref_b] + b_prop)        # [M, 64]
    conv = segment_sum(S, backref, num_segments=N)            # [N, 64]
    out = relu(XW + XW_back * conv + b)

Strategy (8 NeuronCores, SPMD single program):
  - Entries partitioned by owner core of backref (250k pair-entries/core).
  - Tables and accumulators live in DRAM with a (p, w) permuted layout:
    physical row = (logical % 128) * 197 + logical // 128, so phase-1/3
    DMAs are long contiguous per-partition descriptors.  Column w=196 is
    a zero page (gather pads) / trash page (scatter pads).
  - Phase 0: 8 f32 DRAM accumulators zeroed via SWDGE (overlaps phase 1).
  - Phase 1: each core builds the full gather table T = X@W_prop
    (+b_prop/2 folded per row via a ones column); X loads ride the SP
    HWDGE ring, table writes (staged 28 windows at a time) the ACT ring,
    PSUM drain via DVE.  Index loads for early groups are interleaved.
  - Phase 2: entries lex-sorted by (bucket(a), bucket(b)) -> 64 groups,
    within a group ordered by color (= occurrence rank of the backref
    row), runs padded to 128-multiples with per-(group,color) caps =
    max over cores.  Per group: two dma_gathers (int16 indices), DVE
    pair-add, ACT relu, then per-color dma_scatter_add (CCE f32
    accumulate) into one of 8 rotating DRAM accumulators; row-unique
    indices within every scatter op keep the accumulation exact.
    Greedy per-queue descriptor balancing over the 4 SWDGE queues.
  - Phase 3: read back + sum the 8 accumulators per 7-window batch,
    compute XW / XW_back on PE, emit relu(XW + XW_back*conv + b).
  Measured rates that drove the design: SWDGE gather ~2.0 ns/desc and
  HBM CCE scatter ~2.9 ns/desc (4 queues, 8 chains) vs 9-11 ns/desc for
  the v3 SBUF parity scatter; desc generation, not HBM bandwidth, is
  the binding resource at 256 B payloads.
"""
import numpy as np

from concourse import bass, mybir, bacc, tile
from concourse.library_config import mlp

N_CORES = 8
D = 64
CH = 128
RPC = 25088                    # output rows per core (196 windows)
NW = 196                       # windows per core
NWT = NW + 1                   # + zero/trash column
BUCKET = 25088                 # table bucket rows
TROWS = CH * NWT               # 25216 physical rows per bucket / accum
NPAD = N_CORES * RPC           # 200704 padded X rows
NGRP = 64
NACC = 4                       # rotating DRAM accumulators
NQ = 4                         # SWDGE queues
TB1 = 7                        # phase-1 windows per sub-batch
SB1 = 4                        # sub-batches per table write (28 windows)
NB1 = NPAD // (TB1 * CH)       # 224 sub-batches, 56 super-batches
TB3 = 7                        # phase-3 windows per batch
NB3 = NW // TB3                # 28
ZCH = 3152                     # accum zero-write chunk (f32 cols)
PREF = 8                       # phase-2 idx prefetch depth (groups)
LAG = 3                        # gather -> process pipeline lag
SC_W = 1.4                     # scatter desc weight vs gather (queue bal)

# group visit order: ready as soon as buckets <= max(ja, jb) are built
G_ORDER = sorted(range(NGRP), key=lambda g: (max(g >> 3, g & 7), g))

_cache = {}


# ----------------------------------------------------------------------------
# host-side prep
# ----------------------------------------------------------------------------

def _phi(local_rows):
    """bucket-local logical row -> permuted physical row"""
    return (local_rows % CH) * NWT + local_rows // CH


def _wrap16(flat):
    """[n] int16 -> [128, n//16] wrapped layout (16-partition blocks, x8)."""
    n = flat.shape[0]
    w = flat.reshape(n // 16, 16).T.astype(np.int16)
    return np.tile(w, (8, 1))


def _host_prep(X, ref_a, ref_b, backref, W, W_back, W_prop, b, b_prop):
    N = X.shape[0]
    assert N == 200000

    X_pad = np.zeros((NPAD, D + 1), np.float32)
    X_pad[:N, :D] = np.asarray(X, np.float32)
    X_pad[:N, D] = 1.0
    xT = np.ascontiguousarray(X_pad.T)                     # [65, NPAD]

    Wf = np.asarray(W, np.float32)
    Wbk = np.asarray(W_back, np.float32)
    Wp = np.asarray(W_prop, np.float32)
    bf = np.asarray(b, np.float32)
    bpf = np.asarray(b_prop, np.float32)
    w_main = np.concatenate([Wf, bf[None, :]], axis=0)
    w_back = np.concatenate([Wbk, np.zeros((1, D), np.float32)], axis=0)
    w_prop = np.concatenate([Wp, 0.5 * bpf[None, :]], axis=0)

    order = np.argsort(backref, kind="stable")
    sb = np.asarray(backref)[order].astype(np.int64)
    sa = np.asarray(ref_a)[order].astype(np.int64)
    sbb = np.asarray(ref_b)[order].astype(np.int64)
    core_bounds = np.searchsorted(sb, np.arange(N_CORES + 1) * RPC)

    # pass 1: per-core per-group color-sorted entries + per-(g,c) counts
    per_core = []
    counts_gc = {}
    for c in range(N_CORES):
        seg = slice(core_bounds[c], core_bounds[c + 1])
        a = sa[seg]
        bb = sbb[seg]
        br = (sb[seg] - c * RPC).astype(np.int64)
        ja = a // BUCKET
        jb = bb // BUCKET
        la = (a - ja * BUCKET).astype(np.int64)
        lb = (bb - jb * BUCKET).astype(np.int64)
        g = ja * 8 + jb
        gorder = np.argsort(g, kind="stable")
        gs_, la_, lb_, br_ = g[gorder], la[gorder], lb[gorder], br[gorder]
        counts = np.bincount(gs_, minlength=NGRP)
        starts = np.concatenate([[0], np.cumsum(counts)])
        groups = {}
        for gi in range(NGRP):
            sl = slice(starts[gi], starts[gi] + counts[gi])
            ga_, gb_, gr_ = la_[sl], lb_[sl], br_[sl]
            corder = np.argsort(gr_, kind="stable")
            gr_s = gr_[corder]
            n_g = len(gr_s)
            is_new = np.ones(n_g, bool)
            is_new[1:] = gr_s[1:] != gr_s[:-1]
            run_start = np.maximum.accumulate(
                np.where(is_new, np.arange(n_g), 0))
            color_s = np.arange(n_g) - run_start
            color = np.empty(n_g, np.int64)
            color[corder] = color_s
            ccounts = np.bincount(color) if n_g else np.zeros(0, np.int64)
            for ci, k in enumerate(ccounts):
                key = (gi, ci)
                counts_gc[key] = max(counts_gc.get(key, 0), int(k))
            groups[gi] = (ga_, gb_, gr_, color)
        per_core.append(groups)

    # shared run layout (identical across cores): caps padded to 128
    n_colors = max(ci for (_, ci) in counts_gc) + 1
    run_layout = []              # (g, runs) in G_ORDER
    group_meta = []              # per g in G_ORDER: (off, gs, runs)
    off = 0
    for g in G_ORDER:
        g_off = off
        runs = []
        for ci in range(n_colors):
            k = counts_gc.get((g, ci), 0)
            if k == 0:
                continue
            cap = -(-k // CH) * CH
            runs.append((ci, off - g_off, cap))
            off += cap
        group_meta.append((g_off, off - g_off, runs))
        run_layout.append((g, runs))
    gs_tot = off
    assert gs_tot % CH == 0

    pad_rows = (np.arange(gs_tot) % CH) * NWT + NW       # p*197+196

    in_maps = []
    for c in range(N_CORES):
        a_idx = np.empty(gs_tot, np.int16)
        b_idx = np.empty(gs_tot, np.int16)
        s_idx = np.empty(gs_tot, np.int16)
        a_idx[:] = pad_rows
        b_idx[:] = pad_rows
        s_idx[:] = pad_rows
        for (g, runs), (g_off, _, _) in zip(run_layout, group_meta):
            ga_, gb_, gr_, color = per_core[c][g]
            for ci, r_off, cap in runs:
                m = color == ci
                k = int(m.sum())
                assert k <= cap
                base = g_off + r_off
                a_idx[base:base + k] = _phi(ga_[m])
                b_idx[base:base + k] = _phi(gb_[m])
                s_idx[base:base + k] = _phi(gr_[m])

        in_maps.append({
            "xT": xT,
            "xT_own": np.ascontiguousarray(xT[:, c * RPC:(c + 1) * RPC]),
            "w_main": w_main,
            "w_back": w_back,
            "w_prop": w_prop,
            "aidx": np.ascontiguousarray(_wrap16(a_idx)),
            "bidx": np.ascontiguousarray(_wrap16(b_idx)),
            "sidx": np.ascontiguousarray(_wrap16(s_idx)),
        })

    cfg = dict(N=N, gs_tot=gs_tot, group_meta=tuple(
        (g_off, gs, tuple(runs))
        for (g_off, gs, runs) in group_meta))
    return in_maps, cfg


# ----------------------------------------------------------------------------
# device program
# ----------------------------------------------------------------------------

def _build_program(cfg, level=5):
    # level gates (for phase attribution benchmarks): 1=phases0+1,
    # 2=+gathers, 3=+add/relu, 4=+scatter, 5=full
    f32 = mybir.dt.float32
    i16 = mybir.dt.int16
    gs_tot = cfg["gs_tot"]
    group_meta = cfg["group_meta"]

    nc = bacc.Bacc("TRN2", target_bir_lowering=False, debug=False,
                   num_devices=N_CORES, num_swdge_queues=NQ)

    xT = nc.dram_tensor("xT", [D + 1, NPAD], f32, kind="ExternalInput").ap()
    xT_own = nc.dram_tensor("xT_own", [D + 1, RPC], f32,
                            kind="ExternalInput").ap()
    w_main = nc.dram_tensor("w_main", [D + 1, D], f32, kind="ExternalInput").ap()
    w_back = nc.dram_tensor("w_back", [D + 1, D], f32, kind="ExternalInput").ap()
    w_prop = nc.dram_tensor("w_prop", [D + 1, D], f32, kind="ExternalInput").ap()
    aidx = nc.dram_tensor("aidx", [CH, gs_tot // 16], i16,
                          kind="ExternalInput").ap()
    bidx = nc.dram_tensor("bidx", [CH, gs_tot // 16], i16,
                          kind="ExternalInput").ap()
    sidx = nc.dram_tensor("sidx", [CH, gs_tot // 16], i16,
                          kind="ExternalInput").ap()
    tables = [nc.dram_tensor(f"table{j}", [TROWS, D], f32).ap()
              for j in range(N_CORES)]
    accs = [nc.dram_tensor(f"acc{k}", [TROWS, D], f32).ap()
            for k in range(NACC)]
    outp = nc.dram_tensor("out", [RPC, D], f32, kind="ExternalOutput").ap()

    with tile.TileContext(nc) as tc:
        with (
            tc.tile_pool(name="wp", bufs=1) as wp,
            tc.tile_pool(name="zp", bufs=1) as zp,
            tc.tile_pool(name="xp", bufs=3) as xp,
            tc.tile_pool(name="stg", bufs=2) as stg,
            tc.tile_pool(name="idxp", bufs=PREF + 2) as idxp,
            tc.tile_pool(name="gp", bufs=LAG + 1) as gp,
            tc.tile_pool(name="svp", bufs=LAG + 1) as svp,
            tc.tile_pool(name="cvp", bufs=2) as cvp,
            tc.tile_pool(name="op", bufs=3) as op,
            tc.tile_pool(name="ps1", bufs=4, space="PSUM") as ps1,
            tc.tile_pool(name="ps3", bufs=2, space="PSUM") as ps3,
        ):
            nc.gpsimd.load_library(mlp)

            wm_t = wp.tile([D + 1, D], f32)
            wb_t = wp.tile([D + 1, D], f32)
            wpr_t = wp.tile([D + 1, D], f32)
            nc.sync.dma_start(out=wm_t[:], in_=w_main[:])
            nc.sync.dma_start(out=wb_t[:], in_=w_back[:])
            nc.sync.dma_start(out=wpr_t[:], in_=w_prop[:])

            # ---------------- phase 0: zero accs + table pad col (SWDGE) ----
            zt = zp.tile([CH, ZCH], f32)
            nc.vector.memset(zt[:], 0.0)
            zchunks = []
            pos = 0
            while pos < NWT:
                k = min(ZCH // D, NWT - pos)
                zchunks.append((pos, k))
                pos += k
            for k in range(NACC):
                a3 = accs[k][:].rearrange("(p w) d -> p w d", p=CH)
                for (w0, kw) in zchunks:
                    nc.gpsimd.dma_start(out=a3[:, w0:w0 + kw, :],
                                        in_=zt[:, :kw * D].rearrange(
                                            "p (w d) -> p w d", d=D))
            for j in range(N_CORES):
                t3 = tables[j][:].rearrange("(p w) d -> p w d", p=CH)
                nc.gpsimd.dma_start(out=t3[:, NW:NWT, :],
                                    in_=zt[:, :D].rearrange(
                                        "p (w d) -> p w d", d=D))

            # phase-2 idx tiles, prefetched PREF groups ahead
            idx_tiles = {}

            def load_idx(gi_ord):
                g_off, gs, _ = group_meta[gi_ord]
                gw0, gw1 = g_off // 16, (g_off + gs) // 16
                at = idxp.tile([CH, gs // 16], i16, tag="ai")
                bt_ = idxp.tile([CH, gs // 16], i16, tag="bi")
                st_ = idxp.tile([CH, gs // 16], i16, tag="si")
                nc.sync.dma_start(out=at[:], in_=aidx[:, gw0:gw1])
                nc.sync.dma_start(out=bt_[:], in_=bidx[:, gw0:gw1])
                nc.sync.dma_start(out=st_[:], in_=sidx[:, gw0:gw1])
                idx_tiles[gi_ord] = (at, bt_, st_)

            # ---------------- phase 1: build permuted bucket tables ---------
            # X loads split across both HWDGE rings; table writes ride SWDGE
            # (Pool is otherwise idle until the gathers start).
            for sb_ in range(NB1 // SB1):
                j = sb_ // (NB1 // SB1 // N_CORES)
                w0s = (sb_ % (NB1 // SB1 // N_CORES)) * (SB1 * TB1)
                st_big = stg.tile([CH, SB1 * TB1 * D], f32, tag="st")
                for q in range(SB1):
                    bt = sb_ * SB1 + q
                    c0 = bt * TB1 * CH
                    xb = xp.tile([D + 1, TB1 * CH], f32, tag="xb")
                    eng = nc.sync if bt % 2 == 0 else nc.scalar
                    eng.dma_start(out=xb[:], in_=xT[:, c0:c0 + TB1 * CH])
                    ps = ps1.tile([CH, TB1 * D], f32, tag="psA", space="PSUM")
                    for k in range(TB1):
                        nc.tensor.matmul(
                            out=ps[:, k * D:(k + 1) * D],
                            lhsT=xb[:, k * CH:(k + 1) * CH],
                            rhs=wpr_t[:],
                            start=True, stop=True,
                        )
                    nc.vector.tensor_copy(
                        st_big[:, q * TB1 * D:(q + 1) * TB1 * D], ps[:])
                t3 = tables[j][:].rearrange("(p w) d -> p w d", p=CH)
                nc.gpsimd.dma_start(
                    out=t3[:, w0s:w0s + SB1 * TB1, :],
                    in_=st_big[:].rearrange("p (k d) -> p k d", d=D),
                )
                if sb_ < PREF:
                    load_idx(sb_)

            # ---------------- phase 2: gather / pair / relu / scatter -------
            qload = [0.0] * NQ

            def pick_q(weight):
                qi = min(range(NQ), key=lambda i: qload[i])
                qload[qi] += weight
                return qi

            s_rr = 0
            pend = {}
            for gi_ord in range(NGRP + LAG if level >= 2 else 0):
                if gi_ord < NGRP:
                    if gi_ord + PREF < NGRP:
                        load_idx(gi_ord + PREF)
                    g = G_ORDER[gi_ord]
                    g_off, gs, runs = group_meta[gi_ord]
                    ja, jb = g >> 3, g & 7
                    at, bt_, st_ = idx_tiles.pop(gi_ord)
                    ga = gp.tile([CH, (gs // CH) * D], f32, tag="ga")
                    gb = gp.tile([CH, (gs // CH) * D], f32, tag="gb")
                    nc.gpsimd.dma_gather(
                        ga[:].rearrange("p (c d) -> p c d", d=D),
                        tables[ja][:],
                        at[:], gs, gs, D,
                        single_packet=False, queue_num=pick_q(gs),
                    )
                    nc.gpsimd.dma_gather(
                        gb[:].rearrange("p (c d) -> p c d", d=D),
                        tables[jb][:],
                        bt_[:], gs, gs, D,
                        single_packet=False, queue_num=pick_q(gs),
                    )
                    pend[gi_ord] = (ga, gb, st_, group_meta[gi_ord])
                if gi_ord >= LAG:
                    gd = gi_ord - LAG
                    ga, gb, st_, (g_off, gs, runs) = pend.pop(gd)
                    if level < 3:
                        continue
                    nc.vector.tensor_add(ga[:], ga[:], gb[:])
                    sv = svp.tile([CH, (gs // CH) * D], f32, tag="sv")
                    nc.scalar.activation(sv[:], ga[:],
                                         mybir.ActivationFunctionType.Relu)
                    if level < 4:
                        continue
                    s3d = sv[:].rearrange("p (c d) -> p c d", d=D)
                    for ci, r_off, cap in runs:
                        c0, cn = r_off // CH, cap // CH
                        nc.gpsimd.dma_scatter_add(
                            accs[s_rr % NACC][:],
                            s3d[:, c0:c0 + cn, :],
                            st_[:, c0 * 8:(c0 + cn) * 8],
                            cap, cap, D,
                            single_packet=False,
                            queue_num=pick_q(cap * SC_W),
                        )
                        s_rr += 1

            # ---------------- phase 3: combine -----------------------------
            for b3 in range(NB3 if level >= 5 else 0):
                w0 = b3 * TB3
                xb = xp.tile([D + 1, TB3 * CH], f32, tag="xb3")
                nc.scalar.dma_start(
                    out=xb[:], in_=xT_own[:, w0 * CH:(w0 + TB3) * CH])
                psw = ps3.tile([CH, TB3 * D], f32, tag="psw", space="PSUM")
                psb = ps3.tile([CH, TB3 * D], f32, tag="psb", space="PSUM")
                for k in range(TB3):
                    nc.tensor.matmul(
                        out=psw[:, k * D:(k + 1) * D],
                        lhsT=xb[:, k * CH:(k + 1) * CH],
                        rhs=wm_t[:], start=True, stop=True,
                    )
                    nc.tensor.matmul(
                        out=psb[:, k * D:(k + 1) * D],
                        lhsT=xb[:, k * CH:(k + 1) * CH],
                        rhs=wb_t[:], start=True, stop=True,
                    )
                cts = []
                half = max(1, NACC // 2)
                for k in range(NACC):
                    ct = cvp.tile([CH, TB3 * D], f32, tag=f"ct{k % 4}")
                    a3 = accs[k][:].rearrange("(p w) d -> p w d", p=CH)
                    eng = nc.sync if k < half else nc.scalar
                    eng.dma_start(
                        out=ct[:].rearrange("p (w d) -> p w d", d=D),
                        in_=a3[:, w0:w0 + TB3, :])
                    cts.append(ct)
                for k in range(1, half):
                    nc.vector.tensor_add(cts[0][:], cts[0][:], cts[k][:])
                for k in range(half + 1, NACC):
                    nc.vector.tensor_add(cts[half][:], cts[half][:],
                                         cts[k][:])
                if NACC > half:
                    nc.vector.tensor_add(cts[0][:], cts[0][:], cts[half][:])
                t2 = op.tile([CH, TB3 * D], f32, tag="t2")
                nc.vector.tensor_mul(t2[:], psb[:], cts[0][:])
                nc.vector.tensor_add(t2[:], t2[:], psw[:])
                o = op.tile([CH, TB3 * D], f32, tag="o")
                nc.scalar.activation(o[:], t2[:],
                                     mybir.ActivationFunctionType.Relu)
                nc.sync.dma_start(
                    out=outp[w0 * CH:(w0 + TB3) * CH, :].rearrange(
                        "(k p) d -> p k d", p=CH),
                    in_=o[:].rearrange("p (k d) -> p k d", d=D))
            if level < 5:
                o = op.tile([CH, D], f32, tag="oz")
                nc.vector.memset(o[:], 1.0)
                nc.scalar.dma_start(
                    out=outp[:CH, :].rearrange("(k p) d -> p k d", p=CH),
                    in_=o[:].rearrange("p (k d) -> p k d", d=D))

    nc.compile()
    return nc


# ----------------------------------------------------------------------------
# SPMD runner (device-resident inputs, PJRT under axon)
# ----------------------------------------------------------------------------

class SpmdRunner:
    def __init__(self, nc, n_cores=N_CORES):
        import jax
        from jax.sharding import Mesh, PartitionSpec
        from jax.experimental.shard_map import shard_map
        from concourse.bass2jax import (
            install_neuronx_cc_hook, _bass_exec_p, partition_id_tensor)

        install_neuronx_cc_hook()
        self.jax = jax
        self.nc = nc
        self.n_cores = n_cores
        partition_name = (nc.partition_id_tensor.name
                          if nc.partition_id_tensor else None)

        in_names, out_names, out_avals, zero_shapes = [], [], [], []
        for alloc in nc.m.functions[0].allocations:
            if not isinstance(alloc, mybir.MemoryLocationSet):
                continue
            name = alloc.memorylocations[0].name
            if alloc.kind == "ExternalInput":
                if name != partition_name and (
                        nc.dbg_addr is None or name != nc.dbg_addr.name):
                    in_names.append(name)
            elif alloc.kind == "ExternalOutput":
                out_names.append(name)
                shape = tuple(alloc.tensor_shape)
                dtype = mybir.dt.np(alloc.dtype)
                out_avals.append(jax.core.ShapedArray(shape, dtype))
                zero_shapes.append((shape, dtype))
        self.in_names, self.out_names = in_names, out_names
        self.out_avals, self.zero_shapes = out_avals, zero_shapes
        n_params, n_outs = len(in_names), len(out_names)

        all_in_names = list(in_names) + list(out_names)
        dbg_name = nc.dbg_addr.name if nc.dbg_addr is not None else None
        if dbg_name is not None:
            all_in_names.append(dbg_name)
        if partition_name is not None:
            all_in_names.append(partition_name)

        def _body(*args):
            operands = list(args)
            if dbg_name is not None:
                operands.append(jax.numpy.zeros((1, 2), jax.numpy.uint32))
            if partition_name is not None:
                operands.append(partition_id_tensor())
            outs = _bass_exec_p.bind(
                *operands,
                out_avals=tuple(out_avals),
                in_names=tuple(all_in_names),
                out_names=tuple(out_names),
                lowering_input_output_aliases=(),
                sim_require_finite=True,
                sim_require_nnan=True,
                nc=nc,
            )
            return tuple(outs)

        devices = jax.devices()[:n_cores]
        self.mesh = Mesh(np.asarray(devices), ("core",))
        self.pspec = PartitionSpec("core")
        in_specs = (self.pspec,) * (n_params + n_outs)
        out_specs = (self.pspec,) * n_outs
        self.fn = jax.jit(
            shard_map(_body, mesh=self.mesh, in_specs=in_specs,
                      out_specs=out_specs, check_rep=False),
            donate_argnums=tuple(range(n_params, n_params + n_outs)),
            keep_unused=True,
        )

    def put_inputs(self, in_maps):
        sharding = self.jax.sharding.NamedSharding(self.mesh, self.pspec)
        return [
            self.jax.device_put(
                np.concatenate([np.asarray(m[name]) for m in in_maps], axis=0),
                sharding)
            for name in self.in_names
        ]

    def _zeros(self):
        sharding = self.jax.sharding.NamedSharding(self.mesh, self.pspec)
        return [
            self.jax.device_put(
                np.zeros((self.n_cores * s[0], *s[1:]), d), sharding)
            for (s, d) in self.zero_shapes
        ]

    def run(self, dev_in):
        outs = self.fn(*dev_in, *self._zeros())
        self.jax.block_until_ready(outs)
        return outs

    def results(self, outs):
        res = []
        for c in range(self.n_cores):
            d = {}
            for i, name in enumerate(self.out_names):
                shp = self.out_avals[i].shape
                d[name] = np.asarray(outs[i]).reshape(
                    self.n_cores, *shp)[c]
            res.append(d)
        return res


def kernel(X, ref_a, ref_b, backref, e_map, v_count, W, W_back, W_prop,
           b, b_prop, **_unused):
    X = np.asarray(X)
    in_maps, cfg = _host_prep(X, ref_a, ref_b, backref,
                              W, W_back, W_prop, b, b_prop)
    key = ("v5", cfg["N"], cfg["gs_tot"], hash(cfg["group_meta"]))
    if key not in _cache:
        nc = _build_program(cfg)
        _cache[key] = SpmdRunner(nc)
    runner = _cache[key]
    dev_in = runner.put_inputs(in_maps)
    outs = runner.run(dev_in)
    res = runner.results(outs)
    full = np.concatenate([res[c]["out"] for c in range(N_CORES)], axis=0)
    return full[:cfg["N"]].astype(np.float32)


# revision 16
# speedup vs baseline: 1.0014x; 1.0014x over previous
"""Trainium2 Bass kernel v5 for CWL2GCNLayer (WL2 GNN message passing).

reference:
    XW = X @ W; XW_prop = X @ W_prop; XW_back = X @ W_back
    S = relu(XW_prop[ref_a] + XW_prop[ref_b] + b_prop)        # [M, 64]
    conv = segment_sum(S, backref, num_segments=N)            # [N, 64]
    out = relu(XW + XW_back * conv + b)

Strategy (8 NeuronCores, SPMD single program):
  - Entries partitioned by owner core of backref (250k pair-entries/core).
  - Tables and accumulators live in DRAM with a (p, w) permuted layout:
    physical row = (logical % 128) * 197 + logical // 128, so phase-1/3
    DMAs are long contiguous per-partition descriptors.  Column w=196 is
    a zero page (gather pads) / trash page (scatter pads).
  - Phase 0: 8 f32 DRAM accumulators zeroed via SWDGE (overlaps phase 1).
  - Phase 1: each core builds the full gather table T = X@W_prop
    (+b_prop/2 folded per row via a ones column); X loads ride the SP
    HWDGE ring, table writes (staged 28 windows at a time) the ACT ring,
    PSUM drain via DVE.  Index loads for early groups are interleaved.
  - Phase 2: entries lex-sorted by (bucket(a), bucket(b)) -> 64 groups,
    within a group ordered by color (= occurrence rank of the backref
    row), runs padded to 128-multiples with per-(group,color) caps =
    max over cores.  Per group: two dma_gathers (int16 indices), DVE
    pair-add, ACT relu, then per-color dma_scatter_add (CCE f32
    accumulate) into one of 8 rotating DRAM accumulators; row-unique
    indices within every scatter op keep the accumulation exact.
    Greedy per-queue descriptor balancing over the 4 SWDGE queues.
  - Phase 3: read back + sum the 8 accumulators per 7-window batch,
    compute XW / XW_back on PE, emit relu(XW + XW_back*conv + b).
  Measured rates that drove the design: SWDGE gather ~2.0 ns/desc and
  HBM CCE scatter ~2.9 ns/desc (4 queues, 8 chains) vs 9-11 ns/desc for
  the v3 SBUF parity scatter; desc generation, not HBM bandwidth, is
  the binding resource at 256 B payloads.
"""
import numpy as np

from concourse import bass, mybir, bacc, tile
from concourse.library_config import mlp

N_CORES = 8
D = 64
CH = 128
RPC = 25088                    # output rows per core (196 windows)
NW = 196                       # windows per core
NWT = NW + 1                   # + zero/trash column
BUCKET = 25088                 # table bucket rows
TROWS = CH * NWT               # 25216 physical rows per bucket / accum
NPAD = N_CORES * RPC           # 200704 padded X rows
NGRP = 64
NACC = 8                       # rotating DRAM accumulators
NQ = 4                         # SWDGE queues
TB1 = 7                        # phase-1 windows per sub-batch
SB1 = 4                        # sub-batches per table write (28 windows)
NB1 = NPAD // (TB1 * CH)       # 224 sub-batches, 56 super-batches
TB3 = 7                        # phase-3 windows per batch
NB3 = NW // TB3                # 28
ZCH = 3152                     # accum zero-write chunk (f32 cols)
PREF = 8                       # phase-2 idx prefetch depth (groups)
LAG = 3                        # gather -> process pipeline lag
SC_W = 2.0                     # scatter desc weight vs gather (queue bal)

# group visit order: ready as soon as buckets <= max(ja, jb) are built
G_ORDER = sorted(range(NGRP), key=lambda g: (max(g >> 3, g & 7), g))

_cache = {}


# ----------------------------------------------------------------------------
# host-side prep
# ----------------------------------------------------------------------------

def _phi(local_rows):
    """bucket-local logical row -> permuted physical row"""
    return (local_rows % CH) * NWT + local_rows // CH


def _wrap16(flat):
    """[n] int16 -> [128, n//16] wrapped layout (16-partition blocks, x8)."""
    n = flat.shape[0]
    w = flat.reshape(n // 16, 16).T.astype(np.int16)
    return np.tile(w, (8, 1))


def _host_prep(X, ref_a, ref_b, backref, W, W_back, W_prop, b, b_prop):
    N = X.shape[0]
    assert N == 200000

    X_pad = np.zeros((NPAD, D + 1), np.float32)
    X_pad[:N, :D] = np.asarray(X, np.float32)
    X_pad[:N, D] = 1.0
    xT = np.ascontiguousarray(X_pad.T)                     # [65, NPAD]

    Wf = np.asarray(W, np.float32)
    Wbk = np.asarray(W_back, np.float32)
    Wp = np.asarray(W_prop, np.float32)
    bf = np.asarray(b, np.float32)
    bpf = np.asarray(b_prop, np.float32)
    w_main = np.concatenate([Wf, bf[None, :]], axis=0)
    w_back = np.concatenate([Wbk, np.zeros((1, D), np.float32)], axis=0)
    w_prop = np.concatenate([Wp, 0.5 * bpf[None, :]], axis=0)

    order = np.argsort(backref, kind="stable")
    sb = np.asarray(backref)[order].astype(np.int64)
    sa = np.asarray(ref_a)[order].astype(np.int64)
    sbb = np.asarray(ref_b)[order].astype(np.int64)
    core_bounds = np.searchsorted(sb, np.arange(N_CORES + 1) * RPC)

    # pass 1: per-core per-group color-sorted entries + per-(g,c) counts
    per_core = []
    counts_gc = {}
    for c in range(N_CORES):
        seg = slice(core_bounds[c], core_bounds[c + 1])
        a = sa[seg]
        bb = sbb[seg]
        br = (sb[seg] - c * RPC).astype(np.int64)
        ja = a // BUCKET
        jb = bb // BUCKET
        la = (a - ja * BUCKET).astype(np.int64)
        lb = (bb - jb * BUCKET).astype(np.int64)
        g = ja * 8 + jb
        gorder = np.argsort(g, kind="stable")
        gs_, la_, lb_, br_ = g[gorder], la[gorder], lb[gorder], br[gorder]
        counts = np.bincount(gs_, minlength=NGRP)
        starts = np.concatenate([[0], np.cumsum(counts)])
        groups = {}
        for gi in range(NGRP):
            sl = slice(starts[gi], starts[gi] + counts[gi])
            ga_, gb_, gr_ = la_[sl], lb_[sl], br_[sl]
            corder = np.argsort(gr_, kind="stable")
            gr_s = gr_[corder]
            n_g = len(gr_s)
            is_new = np.ones(n_g, bool)
            is_new[1:] = gr_s[1:] != gr_s[:-1]
            run_start = np.maximum.accumulate(
                np.where(is_new, np.arange(n_g), 0))
            color_s = np.arange(n_g) - run_start
            color = np.empty(n_g, np.int64)
            color[corder] = color_s
            ccounts = np.bincount(color) if n_g else np.zeros(0, np.int64)
            for ci, k in enumerate(ccounts):
                key = (gi, ci)
                counts_gc[key] = max(counts_gc.get(key, 0), int(k))
            groups[gi] = (ga_, gb_, gr_, color)
        per_core.append(groups)

    # shared run layout (identical across cores): caps padded to 128
    n_colors = max(ci for (_, ci) in counts_gc) + 1
    run_layout = []              # (g, runs) in G_ORDER
    group_meta = []              # per g in G_ORDER: (off, gs, runs)
    off = 0
    for g in G_ORDER:
        g_off = off
        runs = []
        for ci in range(n_colors):
            k = counts_gc.get((g, ci), 0)
            if k == 0:
                continue
            cap = -(-k // CH) * CH
            runs.append((ci, off - g_off, cap))
            off += cap
        group_meta.append((g_off, off - g_off, runs))
        run_layout.append((g, runs))
    gs_tot = off
    assert gs_tot % CH == 0

    pad_rows = (np.arange(gs_tot) % CH) * NWT + NW       # p*197+196

    in_maps = []
    for c in range(N_CORES):
        a_idx = np.empty(gs_tot, np.int16)
        b_idx = np.empty(gs_tot, np.int16)
        s_idx = np.empty(gs_tot, np.int16)
        a_idx[:] = pad_rows
        b_idx[:] = pad_rows
        s_idx[:] = pad_rows
        for (g, runs), (g_off, _, _) in zip(run_layout, group_meta):
            ga_, gb_, gr_, color = per_core[c][g]
            for ci, r_off, cap in runs:
                m = color == ci
                k = int(m.sum())
                assert k <= cap
                base = g_off + r_off
                a_idx[base:base + k] = _phi(ga_[m])
                b_idx[base:base + k] = _phi(gb_[m])
                s_idx[base:base + k] = _phi(gr_[m])

        in_maps.append({
            "xT": xT,
            "xT_own": np.ascontiguousarray(xT[:, c * RPC:(c + 1) * RPC]),
            "w_main": w_main,
            "w_back": w_back,
            "w_prop": w_prop,
            "aidx": np.ascontiguousarray(_wrap16(a_idx)),
            "bidx": np.ascontiguousarray(_wrap16(b_idx)),
            "sidx": np.ascontiguousarray(_wrap16(s_idx)),
        })

    cfg = dict(N=N, gs_tot=gs_tot, group_meta=tuple(
        (g_off, gs, tuple(runs))
        for (g_off, gs, runs) in group_meta))
    return in_maps, cfg


# ----------------------------------------------------------------------------
# device program
# ----------------------------------------------------------------------------

def _build_program(cfg, level=5):
    # level gates (for phase attribution benchmarks): 1=phases0+1,
    # 2=+gathers, 3=+add/relu, 4=+scatter, 5=full
    f32 = mybir.dt.float32
    i16 = mybir.dt.int16
    gs_tot = cfg["gs_tot"]
    group_meta = cfg["group_meta"]

    nc = bacc.Bacc("TRN2", target_bir_lowering=False, debug=False,
                   num_devices=N_CORES, num_swdge_queues=NQ)

    xT = nc.dram_tensor("xT", [D + 1, NPAD], f32, kind="ExternalInput").ap()
    xT_own = nc.dram_tensor("xT_own", [D + 1, RPC], f32,
                            kind="ExternalInput").ap()
    w_main = nc.dram_tensor("w_main", [D + 1, D], f32, kind="ExternalInput").ap()
    w_back = nc.dram_tensor("w_back", [D + 1, D], f32, kind="ExternalInput").ap()
    w_prop = nc.dram_tensor("w_prop", [D + 1, D], f32, kind="ExternalInput").ap()
    aidx = nc.dram_tensor("aidx", [CH, gs_tot // 16], i16,
                          kind="ExternalInput").ap()
    bidx = nc.dram_tensor("bidx", [CH, gs_tot // 16], i16,
                          kind="ExternalInput").ap()
    sidx = nc.dram_tensor("sidx", [CH, gs_tot // 16], i16,
                          kind="ExternalInput").ap()
    tables = [nc.dram_tensor(f"table{j}", [TROWS, D], f32).ap()
              for j in range(N_CORES)]
    accs = [nc.dram_tensor(f"acc{k}", [TROWS, D], f32).ap()
            for k in range(NACC)]
    outp = nc.dram_tensor("out", [RPC, D], f32, kind="ExternalOutput").ap()

    with tile.TileContext(nc) as tc:
        with (
            tc.tile_pool(name="wp", bufs=1) as wp,
            tc.tile_pool(name="zp", bufs=1) as zp,
            tc.tile_pool(name="xp", bufs=3) as xp,
            tc.tile_pool(name="stg", bufs=2) as stg,
            tc.tile_pool(name="idxp", bufs=PREF + 2) as idxp,
            tc.tile_pool(name="gp", bufs=LAG + 1) as gp,
            tc.tile_pool(name="svp", bufs=LAG + 1) as svp,
            tc.tile_pool(name="cvp", bufs=2) as cvp,
            tc.tile_pool(name="op", bufs=3) as op,
            tc.tile_pool(name="ps1", bufs=4, space="PSUM") as ps1,
            tc.tile_pool(name="ps3", bufs=2, space="PSUM") as ps3,
        ):
            nc.gpsimd.load_library(mlp)

            wm_t = wp.tile([D + 1, D], f32)
            wb_t = wp.tile([D + 1, D], f32)
            wpr_t = wp.tile([D + 1, D], f32)
            nc.sync.dma_start(out=wm_t[:], in_=w_main[:])
            nc.sync.dma_start(out=wb_t[:], in_=w_back[:])
            nc.sync.dma_start(out=wpr_t[:], in_=w_prop[:])

            # ---------------- phase 0: zero accs + table pad col (SWDGE) ----
            zt = zp.tile([CH, ZCH], f32)
            nc.vector.memset(zt[:], 0.0)
            zchunks = []
            pos = 0
            while pos < NWT:
                k = min(ZCH // D, NWT - pos)
                zchunks.append((pos, k))
                pos += k
            for k in range(NACC):
                a3 = accs[k][:].rearrange("(p w) d -> p w d", p=CH)
                for (w0, kw) in zchunks:
                    nc.gpsimd.dma_start(out=a3[:, w0:w0 + kw, :],
                                        in_=zt[:, :kw * D].rearrange(
                                            "p (w d) -> p w d", d=D))
            for j in range(N_CORES):
                t3 = tables[j][:].rearrange("(p w) d -> p w d", p=CH)
                nc.gpsimd.dma_start(out=t3[:, NW:NWT, :],
                                    in_=zt[:, :D].rearrange(
                                        "p (w d) -> p w d", d=D))

            # phase-2 idx tiles, prefetched PREF groups ahead
            idx_tiles = {}

            def load_idx(gi_ord):
                g_off, gs, _ = group_meta[gi_ord]
                gw0, gw1 = g_off // 16, (g_off + gs) // 16
                at = idxp.tile([CH, gs // 16], i16, tag="ai")
                bt_ = idxp.tile([CH, gs // 16], i16, tag="bi")
                st_ = idxp.tile([CH, gs // 16], i16, tag="si")
                nc.sync.dma_start(out=at[:], in_=aidx[:, gw0:gw1])
                nc.sync.dma_start(out=bt_[:], in_=bidx[:, gw0:gw1])
                nc.sync.dma_start(out=st_[:], in_=sidx[:, gw0:gw1])
                idx_tiles[gi_ord] = (at, bt_, st_)

            # ---------------- phase 1: build permuted bucket tables ---------
            # X loads split across both HWDGE rings; table writes ride SWDGE
            # (Pool is otherwise idle until the gathers start).
            for sb_ in range(NB1 // SB1):
                j = sb_ // (NB1 // SB1 // N_CORES)
                w0s = (sb_ % (NB1 // SB1 // N_CORES)) * (SB1 * TB1)
                st_big = stg.tile([CH, SB1 * TB1 * D], f32, tag="st")
                for q in range(SB1):
                    bt = sb_ * SB1 + q
                    c0 = bt * TB1 * CH
                    xb = xp.tile([D + 1, TB1 * CH], f32, tag="xb")
                    eng = nc.sync if bt % 2 == 0 else nc.scalar
                    eng.dma_start(out=xb[:], in_=xT[:, c0:c0 + TB1 * CH])
                    ps = ps1.tile([CH, TB1 * D], f32, tag="psA", space="PSUM")
                    for k in range(TB1):
                        nc.tensor.matmul(
                            out=ps[:, k * D:(k + 1) * D],
                            lhsT=xb[:, k * CH:(k + 1) * CH],
                            rhs=wpr_t[:],
                            start=True, stop=True,
                        )
                    nc.vector.tensor_copy(
                        st_big[:, q * TB1 * D:(q + 1) * TB1 * D], ps[:])
                t3 = tables[j][:].rearrange("(p w) d -> p w d", p=CH)
                nc.gpsimd.dma_start(
                    out=t3[:, w0s:w0s + SB1 * TB1, :],
                    in_=st_big[:].rearrange("p (k d) -> p k d", d=D),
                )
                if sb_ < PREF:
                    load_idx(sb_)

            # ---------------- phase 2: gather / pair / relu / scatter -------
            qload = [0.0] * NQ

            def pick_q(weight):
                qi = min(range(NQ), key=lambda i: qload[i])
                qload[qi] += weight
                return qi

            s_rr = 0
            pend = {}
            for gi_ord in range(NGRP + LAG if level >= 2 else 0):
                if gi_ord < NGRP:
                    if gi_ord + PREF < NGRP:
                        load_idx(gi_ord + PREF)
                    g = G_ORDER[gi_ord]
                    g_off, gs, runs = group_meta[gi_ord]
                    ja, jb = g >> 3, g & 7
                    at, bt_, st_ = idx_tiles.pop(gi_ord)
                    ga = gp.tile([CH, (gs // CH) * D], f32, tag="ga")
                    gb = gp.tile([CH, (gs // CH) * D], f32, tag="gb")
                    nc.gpsimd.dma_gather(
                        ga[:].rearrange("p (c d) -> p c d", d=D),
                        tables[ja][:],
                        at[:], gs, gs, D,
                        single_packet=False, queue_num=pick_q(gs),
                    )
                    nc.gpsimd.dma_gather(
                        gb[:].rearrange("p (c d) -> p c d", d=D),
                        tables[jb][:],
                        bt_[:], gs, gs, D,
                        single_packet=False, queue_num=pick_q(gs),
                    )
                    pend[gi_ord] = (ga, gb, st_, group_meta[gi_ord])
                if gi_ord >= LAG:
                    gd = gi_ord - LAG
                    ga, gb, st_, (g_off, gs, runs) = pend.pop(gd)
                    if level < 3:
                        continue
                    nc.vector.tensor_add(ga[:], ga[:], gb[:])
                    sv = svp.tile([CH, (gs // CH) * D], f32, tag="sv")
                    nc.scalar.activation(sv[:], ga[:],
                                         mybir.ActivationFunctionType.Relu)
                    if level < 4:
                        continue
                    s3d = sv[:].rearrange("p (c d) -> p c d", d=D)
                    for ci, r_off, cap in runs:
                        c0, cn = r_off // CH, cap // CH
                        nc.gpsimd.dma_scatter_add(
                            accs[s_rr % NACC][:],
                            s3d[:, c0:c0 + cn, :],
                            st_[:, c0 * 8:(c0 + cn) * 8],
                            cap, cap, D,
                            single_packet=False,
                            queue_num=pick_q(cap * SC_W),
                        )
                        s_rr += 1

            # ---------------- phase 3: combine -----------------------------
            for b3 in range(NB3 if level >= 5 else 0):
                w0 = b3 * TB3
                xb = xp.tile([D + 1, TB3 * CH], f32, tag="xb3")
                nc.scalar.dma_start(
                    out=xb[:], in_=xT_own[:, w0 * CH:(w0 + TB3) * CH])
                psw = ps3.tile([CH, TB3 * D], f32, tag="psw", space="PSUM")
                psb = ps3.tile([CH, TB3 * D], f32, tag="psb", space="PSUM")
                for k in range(TB3):
                    nc.tensor.matmul(
                        out=psw[:, k * D:(k + 1) * D],
                        lhsT=xb[:, k * CH:(k + 1) * CH],
                        rhs=wm_t[:], start=True, stop=True,
                    )
                    nc.tensor.matmul(
                        out=psb[:, k * D:(k + 1) * D],
                        lhsT=xb[:, k * CH:(k + 1) * CH],
                        rhs=wb_t[:], start=True, stop=True,
                    )
                cts = []
                half = max(1, NACC // 2)
                for k in range(NACC):
                    ct = cvp.tile([CH, TB3 * D], f32, tag=f"ct{k % 4}")
                    a3 = accs[k][:].rearrange("(p w) d -> p w d", p=CH)
                    eng = nc.sync if k < half else nc.scalar
                    eng.dma_start(
                        out=ct[:].rearrange("p (w d) -> p w d", d=D),
                        in_=a3[:, w0:w0 + TB3, :])
                    cts.append(ct)
                for k in range(1, half):
                    nc.vector.tensor_add(cts[0][:], cts[0][:], cts[k][:])
                for k in range(half + 1, NACC):
                    nc.vector.tensor_add(cts[half][:], cts[half][:],
                                         cts[k][:])
                if NACC > half:
                    nc.vector.tensor_add(cts[0][:], cts[0][:], cts[half][:])
                t2 = op.tile([CH, TB3 * D], f32, tag="t2")
                nc.vector.tensor_mul(t2[:], psb[:], cts[0][:])
                nc.vector.tensor_add(t2[:], t2[:], psw[:])
                o = op.tile([CH, TB3 * D], f32, tag="o")
                nc.scalar.activation(o[:], t2[:],
                                     mybir.ActivationFunctionType.Relu)
                nc.sync.dma_start(
                    out=outp[w0 * CH:(w0 + TB3) * CH, :].rearrange(
                        "(k p) d -> p k d", p=CH),
                    in_=o[:].rearrange("p (k d) -> p k d", d=D))
            if level < 5:
                o = op.tile([CH, D], f32, tag="oz")
                nc.vector.memset(o[:], 1.0)
                nc.scalar.dma_start(
                    out=outp[:CH, :].rearrange("(k p) d -> p k d", p=CH),
                    in_=o[:].rearrange("p (k d) -> p k d", d=D))

    nc.compile()
    return nc


# ----------------------------------------------------------------------------
# SPMD runner (device-resident inputs, PJRT under axon)
# ----------------------------------------------------------------------------

class SpmdRunner:
    def __init__(self, nc, n_cores=N_CORES):
        import jax
        from jax.sharding import Mesh, PartitionSpec
        from jax.experimental.shard_map import shard_map
        from concourse.bass2jax import (
            install_neuronx_cc_hook, _bass_exec_p, partition_id_tensor)

        install_neuronx_cc_hook()
        self.jax = jax
        self.nc = nc
        self.n_cores = n_cores
        partition_name = (nc.partition_id_tensor.name
                          if nc.partition_id_tensor else None)

        in_names, out_names, out_avals, zero_shapes = [], [], [], []
        for alloc in nc.m.functions[0].allocations:
            if not isinstance(alloc, mybir.MemoryLocationSet):
                continue
            name = alloc.memorylocations[0].name
            if alloc.kind == "ExternalInput":
                if name != partition_name and (
                        nc.dbg_addr is None or name != nc.dbg_addr.name):
                    in_names.append(name)
            elif alloc.kind == "ExternalOutput":
                out_names.append(name)
                shape = tuple(alloc.tensor_shape)
                dtype = mybir.dt.np(alloc.dtype)
                out_avals.append(jax.core.ShapedArray(shape, dtype))
                zero_shapes.append((shape, dtype))
        self.in_names, self.out_names = in_names, out_names
        self.out_avals, self.zero_shapes = out_avals, zero_shapes
        n_params, n_outs = len(in_names), len(out_names)

        all_in_names = list(in_names) + list(out_names)
        dbg_name = nc.dbg_addr.name if nc.dbg_addr is not None else None
        if dbg_name is not None:
            all_in_names.append(dbg_name)
        if partition_name is not None:
            all_in_names.append(partition_name)

        def _body(*args):
            operands = list(args)
            if dbg_name is not None:
                operands.append(jax.numpy.zeros((1, 2), jax.numpy.uint32))
            if partition_name is not None:
                operands.append(partition_id_tensor())
            outs = _bass_exec_p.bind(
                *operands,
                out_avals=tuple(out_avals),
                in_names=tuple(all_in_names),
                out_names=tuple(out_names),
                lowering_input_output_aliases=(),
                sim_require_finite=True,
                sim_require_nnan=True,
                nc=nc,
            )
            return tuple(outs)

        devices = jax.devices()[:n_cores]
        self.mesh = Mesh(np.asarray(devices), ("core",))
        self.pspec = PartitionSpec("core")
        in_specs = (self.pspec,) * (n_params + n_outs)
        out_specs = (self.pspec,) * n_outs
        self.fn = jax.jit(
            shard_map(_body, mesh=self.mesh, in_specs=in_specs,
                      out_specs=out_specs, check_rep=False),
            donate_argnums=tuple(range(n_params, n_params + n_outs)),
            keep_unused=True,
        )

    def put_inputs(self, in_maps):
        sharding = self.jax.sharding.NamedSharding(self.mesh, self.pspec)
        return [
            self.jax.device_put(
                np.concatenate([np.asarray(m[name]) for m in in_maps], axis=0),
                sharding)
            for name in self.in_names
        ]

    def _zeros(self):
        sharding = self.jax.sharding.NamedSharding(self.mesh, self.pspec)
        return [
            self.jax.device_put(
                np.zeros((self.n_cores * s[0], *s[1:]), d), sharding)
            for (s, d) in self.zero_shapes
        ]

    def run(self, dev_in):
        outs = self.fn(*dev_in, *self._zeros())
        self.jax.block_until_ready(outs)
        return outs

    def results(self, outs):
        res = []
        for c in range(self.n_cores):
            d = {}
            for i, name in enumerate(self.out_names):
                shp = self.out_avals[i].shape
                d[name] = np.asarray(outs[i]).reshape(
                    self.n_cores, *shp)[c]
            res.append(d)
        return res


def kernel(X, ref_a, ref_b, backref, e_map, v_count, W, W_back, W_prop,
           b, b_prop, **_unused):
    X = np.asarray(X)
    in_maps, cfg = _host_prep(X, ref_a, ref_b, backref,
                              W, W_back, W_prop, b, b_prop)
    key = ("v5", cfg["N"], cfg["gs_tot"], hash(cfg["group_meta"]))
    if key not in _cache:
        nc = _build_program(cfg)
        _cache[key] = SpmdRunner(nc)
    runner = _cache[key]
    dev_in = runner.put_inputs(in_maps)
    outs = runner.run(dev_in)
    res = runner.results(outs)
    full = np.concatenate([res[c]["out"] for c in range(N_CORES)], axis=0)
    return full[:cfg["N"]].astype(np.float32)


# revision 17
# speedup vs baseline: 1.0077x; 1.0063x over previous
"""Trainium2 Bass kernel v5 for CWL2GCNLayer (WL2 GNN message passing).

reference:
    XW = X @ W; XW_prop = X @ W_prop; XW_back = X @ W_back
    S = relu(XW_prop[ref_a] + XW_prop[ref_b] + b_prop)        # [M, 64]
    conv = segment_sum(S, backref, num_segments=N)            # [N, 64]
    out = relu(XW + XW_back * conv + b)

Strategy (8 NeuronCores, SPMD single program):
  - Entries partitioned by owner core of backref (250k pair-entries/core).
  - Tables and accumulators live in DRAM with a (p, w) permuted layout:
    physical row = (logical % 128) * 197 + logical // 128, so phase-1/3
    DMAs are long contiguous per-partition descriptors.  Column w=196 is
    a zero page (gather pads) / trash page (scatter pads).
  - Phase 0: 8 f32 DRAM accumulators zeroed via SWDGE (overlaps phase 1).
  - Phase 1: each core builds the full gather table T = X@W_prop
    (+b_prop/2 folded per row via a ones column); X loads ride the SP
    HWDGE ring, table writes (staged 28 windows at a time) the ACT ring,
    PSUM drain via DVE.  Index loads for early groups are interleaved.
  - Phase 2: entries lex-sorted by (bucket(a), bucket(b)) -> 64 groups,
    within a group ordered by color (= occurrence rank of the backref
    row), runs padded to 128-multiples with per-(group,color) caps =
    max over cores.  Per group: two dma_gathers (int16 indices), DVE
    pair-add, ACT relu, then per-color dma_scatter_add (CCE f32
    accumulate) into one of 8 rotating DRAM accumulators; row-unique
    indices within every scatter op keep the accumulation exact.
    Greedy per-queue descriptor balancing over the 4 SWDGE queues.
  - Phase 3: read back + sum the 8 accumulators per 7-window batch,
    compute XW / XW_back on PE, emit relu(XW + XW_back*conv + b).
  Measured rates that drove the design: SWDGE gather ~2.0 ns/desc and
  HBM CCE scatter ~2.9 ns/desc (4 queues, 8 chains) vs 9-11 ns/desc for
  the v3 SBUF parity scatter; desc generation, not HBM bandwidth, is
  the binding resource at 256 B payloads.
"""
import numpy as np

from concourse import bass, mybir, bacc, tile
from concourse.library_config import mlp

N_CORES = 8
D = 64
CH = 128
RPC = 25088                    # output rows per core (196 windows)
NW = 196                       # windows per core
NWT = NW + 1                   # + zero/trash column
BUCKET = 25088                 # table bucket rows
TROWS = CH * NWT               # 25216 physical rows per bucket / accum
NPAD = N_CORES * RPC           # 200704 padded X rows
NGRP = 64
NACC = 8                       # rotating DRAM accumulators
NQ = 4                         # SWDGE queues
TB1 = 7                        # phase-1 windows per sub-batch
SB1 = 4                        # sub-batches per table write (28 windows)
NB1 = NPAD // (TB1 * CH)       # 224 sub-batches, 56 super-batches
TB3 = 7                        # phase-3 windows per batch
NB3 = NW // TB3                # 28
ZCH = 3152                     # accum zero-write chunk (f32 cols)
PREF = 8                       # phase-2 idx prefetch depth (groups)
LAG = 3                        # gather -> process pipeline lag
SC_W = 1.4                     # scatter desc weight vs gather (queue bal)

# group visit order: ready as soon as buckets <= max(ja, jb) are built
G_ORDER = sorted(range(NGRP), key=lambda g: (max(g >> 3, g & 7), g))

_cache = {}


# ----------------------------------------------------------------------------
# host-side prep
# ----------------------------------------------------------------------------

def _phi(local_rows):
    """bucket-local logical row -> permuted physical row"""
    return (local_rows % CH) * NWT + local_rows // CH


def _wrap16(flat):
    """[n] int16 -> [128, n//16] wrapped layout (16-partition blocks, x8)."""
    n = flat.shape[0]
    w = flat.reshape(n // 16, 16).T.astype(np.int16)
    return np.tile(w, (8, 1))


def _host_prep(X, ref_a, ref_b, backref, W, W_back, W_prop, b, b_prop):
    N = X.shape[0]
    assert N == 200000

    X_pad = np.zeros((NPAD, D + 1), np.float32)
    X_pad[:N, :D] = np.asarray(X, np.float32)
    X_pad[:N, D] = 1.0
    xT = np.ascontiguousarray(X_pad.T)                     # [65, NPAD]

    Wf = np.asarray(W, np.float32)
    Wbk = np.asarray(W_back, np.float32)
    Wp = np.asarray(W_prop, np.float32)
    bf = np.asarray(b, np.float32)
    bpf = np.asarray(b_prop, np.float32)
    w_main = np.concatenate([Wf, bf[None, :]], axis=0)
    w_back = np.concatenate([Wbk, np.zeros((1, D), np.float32)], axis=0)
    w_prop = np.concatenate([Wp, 0.5 * bpf[None, :]], axis=0)

    order = np.argsort(backref, kind="stable")
    sb = np.asarray(backref)[order].astype(np.int64)
    sa = np.asarray(ref_a)[order].astype(np.int64)
    sbb = np.asarray(ref_b)[order].astype(np.int64)
    core_bounds = np.searchsorted(sb, np.arange(N_CORES + 1) * RPC)

    # pass 1: per-core per-group color-sorted entries + per-(g,c) counts
    per_core = []
    counts_gc = {}
    for c in range(N_CORES):
        seg = slice(core_bounds[c], core_bounds[c + 1])
        a = sa[seg]
        bb = sbb[seg]
        br = (sb[seg] - c * RPC).astype(np.int64)
        ja = a // BUCKET
        jb = bb // BUCKET
        la = (a - ja * BUCKET).astype(np.int64)
        lb = (bb - jb * BUCKET).astype(np.int64)
        g = ja * 8 + jb
        gorder = np.argsort(g, kind="stable")
        gs_, la_, lb_, br_ = g[gorder], la[gorder], lb[gorder], br[gorder]
        counts = np.bincount(gs_, minlength=NGRP)
        starts = np.concatenate([[0], np.cumsum(counts)])
        groups = {}
        for gi in range(NGRP):
            sl = slice(starts[gi], starts[gi] + counts[gi])
            ga_, gb_, gr_ = la_[sl], lb_[sl], br_[sl]
            corder = np.argsort(gr_, kind="stable")
            gr_s = gr_[corder]
            n_g = len(gr_s)
            is_new = np.ones(n_g, bool)
            is_new[1:] = gr_s[1:] != gr_s[:-1]
            run_start = np.maximum.accumulate(
                np.where(is_new, np.arange(n_g), 0))
            color_s = np.arange(n_g) - run_start
            color = np.empty(n_g, np.int64)
            color[corder] = color_s
            ccounts = np.bincount(color) if n_g else np.zeros(0, np.int64)
            for ci, k in enumerate(ccounts):
                key = (gi, ci)
                counts_gc[key] = max(counts_gc.get(key, 0), int(k))
            groups[gi] = (ga_, gb_, gr_, color)
        per_core.append(groups)

    # shared run layout (identical across cores): caps padded to 128
    n_colors = max(ci for (_, ci) in counts_gc) + 1
    run_layout = []              # (g, runs) in G_ORDER
    group_meta = []              # per g in G_ORDER: (off, gs, runs)
    off = 0
    for g in G_ORDER:
        g_off = off
        runs = []
        for ci in range(n_colors):
            k = counts_gc.get((g, ci), 0)
            if k == 0:
                continue
            cap = -(-k // CH) * CH
            runs.append((ci, off - g_off, cap))
            off += cap
        group_meta.append((g_off, off - g_off, runs))
        run_layout.append((g, runs))
    gs_tot = off
    assert gs_tot % CH == 0

    pad_rows = (np.arange(gs_tot) % CH) * NWT + NW       # p*197+196

    in_maps = []
    for c in range(N_CORES):
        a_idx = np.empty(gs_tot, np.int16)
        b_idx = np.empty(gs_tot, np.int16)
        s_idx = np.empty(gs_tot, np.int16)
        a_idx[:] = pad_rows
        b_idx[:] = pad_rows
        s_idx[:] = pad_rows
        for (g, runs), (g_off, _, _) in zip(run_layout, group_meta):
            ga_, gb_, gr_, color = per_core[c][g]
            for ci, r_off, cap in runs:
                m = color == ci
                k = int(m.sum())
                assert k <= cap
                base = g_off + r_off
                a_idx[base:base + k] = _phi(ga_[m])
                b_idx[base:base + k] = _phi(gb_[m])
                s_idx[base:base + k] = _phi(gr_[m])

        in_maps.append({
            "xT": xT,
            "xT_own": np.ascontiguousarray(xT[:, c * RPC:(c + 1) * RPC]),
            "w_main": w_main,
            "w_back": w_back,
            "w_prop": w_prop,
            "aidx": np.ascontiguousarray(_wrap16(a_idx)),
            "bidx": np.ascontiguousarray(_wrap16(b_idx)),
            "sidx": np.ascontiguousarray(_wrap16(s_idx)),
        })

    cfg = dict(N=N, gs_tot=gs_tot, group_meta=tuple(
        (g_off, gs, tuple(runs))
        for (g_off, gs, runs) in group_meta))
    return in_maps, cfg


# ----------------------------------------------------------------------------
# device program
# ----------------------------------------------------------------------------

def _build_program(cfg, level=5):
    # level gates (for phase attribution benchmarks): 1=phases0+1,
    # 2=+gathers, 3=+add/relu, 4=+scatter, 5=full
    f32 = mybir.dt.float32
    i16 = mybir.dt.int16
    gs_tot = cfg["gs_tot"]
    group_meta = cfg["group_meta"]

    nc = bacc.Bacc("TRN2", target_bir_lowering=False, debug=False,
                   num_devices=N_CORES, num_swdge_queues=NQ)

    xT = nc.dram_tensor("xT", [D + 1, NPAD], f32, kind="ExternalInput").ap()
    xT_own = nc.dram_tensor("xT_own", [D + 1, RPC], f32,
                            kind="ExternalInput").ap()
    w_main = nc.dram_tensor("w_main", [D + 1, D], f32, kind="ExternalInput").ap()
    w_back = nc.dram_tensor("w_back", [D + 1, D], f32, kind="ExternalInput").ap()
    w_prop = nc.dram_tensor("w_prop", [D + 1, D], f32, kind="ExternalInput").ap()
    aidx = nc.dram_tensor("aidx", [CH, gs_tot // 16], i16,
                          kind="ExternalInput").ap()
    bidx = nc.dram_tensor("bidx", [CH, gs_tot // 16], i16,
                          kind="ExternalInput").ap()
    sidx = nc.dram_tensor("sidx", [CH, gs_tot // 16], i16,
                          kind="ExternalInput").ap()
    tables = [nc.dram_tensor(f"table{j}", [TROWS, D], f32).ap()
              for j in range(N_CORES)]
    accs = [nc.dram_tensor(f"acc{k}", [TROWS, D], f32).ap()
            for k in range(NACC)]
    outp = nc.dram_tensor("out", [RPC, D], f32, kind="ExternalOutput").ap()

    with tile.TileContext(nc) as tc:
        with (
            tc.tile_pool(name="wp", bufs=1) as wp,
            tc.tile_pool(name="zp", bufs=1) as zp,
            tc.tile_pool(name="xp", bufs=3) as xp,
            tc.tile_pool(name="stg", bufs=2) as stg,
            tc.tile_pool(name="idxp", bufs=PREF + 2) as idxp,
            tc.tile_pool(name="gp", bufs=LAG + 1) as gp,
            tc.tile_pool(name="svp", bufs=LAG + 1) as svp,
            tc.tile_pool(name="cvp", bufs=2) as cvp,
            tc.tile_pool(name="op", bufs=3) as op,
            tc.tile_pool(name="ps1", bufs=4, space="PSUM") as ps1,
            tc.tile_pool(name="ps3", bufs=2, space="PSUM") as ps3,
        ):
            nc.gpsimd.load_library(mlp)

            wm_t = wp.tile([D + 1, D], f32)
            wb_t = wp.tile([D + 1, D], f32)
            wpr_t = wp.tile([D + 1, D], f32)
            nc.sync.dma_start(out=wm_t[:], in_=w_main[:])
            nc.sync.dma_start(out=wb_t[:], in_=w_back[:])
            nc.sync.dma_start(out=wpr_t[:], in_=w_prop[:])

            # ---------------- phase 0: zero accs + table pad col (SWDGE) ----
            zt = zp.tile([CH, ZCH], f32)
            nc.vector.memset(zt[:], 0.0)
            zchunks = []
            pos = 0
            while pos < NWT:
                k = min(ZCH // D, NWT - pos)
                zchunks.append((pos, k))
                pos += k
            for k in range(NACC):
                a3 = accs[k][:].rearrange("(p w) d -> p w d", p=CH)
                for (w0, kw) in zchunks:
                    nc.gpsimd.dma_start(out=a3[:, w0:w0 + kw, :],
                                        in_=zt[:, :kw * D].rearrange(
                                            "p (w d) -> p w d", d=D))
            for j in range(N_CORES):
                t3 = tables[j][:].rearrange("(p w) d -> p w d", p=CH)
                nc.gpsimd.dma_start(out=t3[:, NW:NWT, :],
                                    in_=zt[:, :D].rearrange(
                                        "p (w d) -> p w d", d=D))

            # phase-2 idx tiles, prefetched PREF groups ahead
            idx_tiles = {}

            def load_idx(gi_ord):
                g_off, gs, _ = group_meta[gi_ord]
                gw0, gw1 = g_off // 16, (g_off + gs) // 16
                at = idxp.tile([CH, gs // 16], i16, tag="ai")
                bt_ = idxp.tile([CH, gs // 16], i16, tag="bi")
                st_ = idxp.tile([CH, gs // 16], i16, tag="si")
                nc.sync.dma_start(out=at[:], in_=aidx[:, gw0:gw1])
                nc.sync.dma_start(out=bt_[:], in_=bidx[:, gw0:gw1])
                nc.sync.dma_start(out=st_[:], in_=sidx[:, gw0:gw1])
                idx_tiles[gi_ord] = (at, bt_, st_)

            # ---------------- phase 1: build permuted bucket tables ---------
            # X loads split across both HWDGE rings; table writes ride SWDGE
            # (Pool is otherwise idle until the gathers start).
            for sb_ in range(NB1 // SB1):
                j = sb_ // (NB1 // SB1 // N_CORES)
                w0s = (sb_ % (NB1 // SB1 // N_CORES)) * (SB1 * TB1)
                st_big = stg.tile([CH, SB1 * TB1 * D], f32, tag="st")
                for q in range(SB1):
                    bt = sb_ * SB1 + q
                    c0 = bt * TB1 * CH
                    xb = xp.tile([D + 1, TB1 * CH], f32, tag="xb")
                    eng = nc.sync if bt % 2 == 0 else nc.scalar
                    eng.dma_start(out=xb[:], in_=xT[:, c0:c0 + TB1 * CH])
                    ps = ps1.tile([CH, TB1 * D], f32, tag="psA", space="PSUM")
                    for k in range(TB1):
                        nc.tensor.matmul(
                            out=ps[:, k * D:(k + 1) * D],
                            lhsT=xb[:, k * CH:(k + 1) * CH],
                            rhs=wpr_t[:],
                            start=True, stop=True,
                        )
                    nc.vector.tensor_copy(
                        st_big[:, q * TB1 * D:(q + 1) * TB1 * D], ps[:])
                t3 = tables[j][:].rearrange("(p w) d -> p w d", p=CH)
                nc.gpsimd.dma_start(
                    out=t3[:, w0s:w0s + SB1 * TB1, :],
                    in_=st_big[:].rearrange("p (k d) -> p k d", d=D),
                )
                if sb_ < PREF:
                    load_idx(sb_)

            # ---------------- phase 2: gather / pair / relu / scatter -------
            qload = [0.0] * NQ

            def pick_q(weight):
                qi = min(range(NQ), key=lambda i: qload[i])
                qload[qi] += weight
                return qi

            s_rr = 0
            pend = {}
            for gi_ord in range(NGRP + LAG if level >= 2 else 0):
                if gi_ord < NGRP:
                    if gi_ord + PREF < NGRP:
                        load_idx(gi_ord + PREF)
                    g = G_ORDER[gi_ord]
                    g_off, gs, runs = group_meta[gi_ord]
                    ja, jb = g >> 3, g & 7
                    at, bt_, st_ = idx_tiles.pop(gi_ord)
                    ga = gp.tile([CH, (gs // CH) * D], f32, tag="ga")
                    gb = gp.tile([CH, (gs // CH) * D], f32, tag="gb")
                    nc.gpsimd.dma_gather(
                        ga[:].rearrange("p (c d) -> p c d", d=D),
                        tables[ja][:],
                        at[:], gs, gs, D,
                        single_packet=False, queue_num=pick_q(gs),
                    )
                    nc.gpsimd.dma_gather(
                        gb[:].rearrange("p (c d) -> p c d", d=D),
                        tables[jb][:],
                        bt_[:], gs, gs, D,
                        single_packet=False, queue_num=pick_q(gs),
                    )
                    pend[gi_ord] = (ga, gb, st_, group_meta[gi_ord])
                if gi_ord >= LAG:
                    gd = gi_ord - LAG
                    ga, gb, st_, (g_off, gs, runs) = pend.pop(gd)
                    if level < 3:
                        continue
                    nc.vector.tensor_add(ga[:], ga[:], gb[:])
                    sv = svp.tile([CH, (gs // CH) * D], f32, tag="sv")
                    nc.scalar.activation(sv[:], ga[:],
                                         mybir.ActivationFunctionType.Relu)
                    if level < 4:
                        continue
                    s3d = sv[:].rearrange("p (c d) -> p c d", d=D)
                    for ci, r_off, cap in runs:
                        c0, cn = r_off // CH, cap // CH
                        nc.gpsimd.dma_scatter_add(
                            accs[s_rr % NACC][:],
                            s3d[:, c0:c0 + cn, :],
                            st_[:, c0 * 8:(c0 + cn) * 8],
                            cap, cap, D,
                            single_packet=False,
                            queue_num=pick_q(cap * SC_W),
                        )
                        s_rr += 1

            # ---------------- phase 3: combine -----------------------------
            for b3 in range(NB3 if level >= 5 else 0):
                w0 = b3 * TB3
                xb = xp.tile([D + 1, TB3 * CH], f32, tag="xb3")
                nc.scalar.dma_start(
                    out=xb[:], in_=xT_own[:, w0 * CH:(w0 + TB3) * CH])
                psw = ps3.tile([CH, TB3 * D], f32, tag="psw", space="PSUM")
                psb = ps3.tile([CH, TB3 * D], f32, tag="psb", space="PSUM")
                for k in range(TB3):
                    nc.tensor.matmul(
                        out=psw[:, k * D:(k + 1) * D],
                        lhsT=xb[:, k * CH:(k + 1) * CH],
                        rhs=wm_t[:], start=True, stop=True,
                    )
                    nc.tensor.matmul(
                        out=psb[:, k * D:(k + 1) * D],
                        lhsT=xb[:, k * CH:(k + 1) * CH],
                        rhs=wb_t[:], start=True, stop=True,
                    )
                cts = []
                half = max(1, NACC // 2)
                for k in range(NACC):
                    ct = cvp.tile([CH, TB3 * D], f32, tag=f"ct{k % 4}")
                    a3 = accs[k][:].rearrange("(p w) d -> p w d", p=CH)
                    eng = nc.sync if k < half else nc.scalar
                    eng.dma_start(
                        out=ct[:].rearrange("p (w d) -> p w d", d=D),
                        in_=a3[:, w0:w0 + TB3, :])
                    cts.append(ct)
                for k in range(1, half):
                    nc.vector.tensor_add(cts[0][:], cts[0][:], cts[k][:])
                for k in range(half + 1, NACC):
                    nc.vector.tensor_add(cts[half][:], cts[half][:],
                                         cts[k][:])
                if NACC > half:
                    nc.vector.tensor_add(cts[0][:], cts[0][:], cts[half][:])
                t2 = op.tile([CH, TB3 * D], f32, tag="t2")
                nc.vector.tensor_mul(t2[:], psb[:], cts[0][:])
                nc.vector.tensor_add(t2[:], t2[:], psw[:])
                o = op.tile([CH, TB3 * D], f32, tag="o")
                nc.scalar.activation(o[:], t2[:],
                                     mybir.ActivationFunctionType.Relu)
                nc.sync.dma_start(
                    out=outp[w0 * CH:(w0 + TB3) * CH, :].rearrange(
                        "(k p) d -> p k d", p=CH),
                    in_=o[:].rearrange("p (k d) -> p k d", d=D))
            if level < 5:
                o = op.tile([CH, D], f32, tag="oz")
                nc.vector.memset(o[:], 1.0)
                nc.scalar.dma_start(
                    out=outp[:CH, :].rearrange("(k p) d -> p k d", p=CH),
                    in_=o[:].rearrange("p (k d) -> p k d", d=D))

    nc.compile()
    return nc


# ----------------------------------------------------------------------------
# SPMD runner (device-resident inputs, PJRT under axon)
# ----------------------------------------------------------------------------

class SpmdRunner:
    def __init__(self, nc, n_cores=N_CORES):
        import jax
        from jax.sharding import Mesh, PartitionSpec
        from jax.experimental.shard_map import shard_map
        from concourse.bass2jax import (
            install_neuronx_cc_hook, _bass_exec_p, partition_id_tensor)

        install_neuronx_cc_hook()
        self.jax = jax
        self.nc = nc
        self.n_cores = n_cores
        partition_name = (nc.partition_id_tensor.name
                          if nc.partition_id_tensor else None)

        in_names, out_names, out_avals, zero_shapes = [], [], [], []
        for alloc in nc.m.functions[0].allocations:
            if not isinstance(alloc, mybir.MemoryLocationSet):
                continue
            name = alloc.memorylocations[0].name
            if alloc.kind == "ExternalInput":
                if name != partition_name and (
                        nc.dbg_addr is None or name != nc.dbg_addr.name):
                    in_names.append(name)
            elif alloc.kind == "ExternalOutput":
                out_names.append(name)
                shape = tuple(alloc.tensor_shape)
                dtype = mybir.dt.np(alloc.dtype)
                out_avals.append(jax.core.ShapedArray(shape, dtype))
                zero_shapes.append((shape, dtype))
        self.in_names, self.out_names = in_names, out_names
        self.out_avals, self.zero_shapes = out_avals, zero_shapes
        n_params, n_outs = len(in_names), len(out_names)

        all_in_names = list(in_names) + list(out_names)
        dbg_name = nc.dbg_addr.name if nc.dbg_addr is not None else None
        if dbg_name is not None:
            all_in_names.append(dbg_name)
        if partition_name is not None:
            all_in_names.append(partition_name)

        def _body(*args):
            operands = list(args)
            if dbg_name is not None:
                operands.append(jax.numpy.zeros((1, 2), jax.numpy.uint32))
            if partition_name is not None:
                operands.append(partition_id_tensor())
            outs = _bass_exec_p.bind(
                *operands,
                out_avals=tuple(out_avals),
                in_names=tuple(all_in_names),
                out_names=tuple(out_names),
                lowering_input_output_aliases=(),
                sim_require_finite=True,
                sim_require_nnan=True,
                nc=nc,
            )
            return tuple(outs)

        devices = jax.devices()[:n_cores]
        self.mesh = Mesh(np.asarray(devices), ("core",))
        self.pspec = PartitionSpec("core")
        in_specs = (self.pspec,) * (n_params + n_outs)
        out_specs = (self.pspec,) * n_outs
        self.fn = jax.jit(
            shard_map(_body, mesh=self.mesh, in_specs=in_specs,
                      out_specs=out_specs, check_rep=False),
            donate_argnums=tuple(range(n_params, n_params + n_outs)),
            keep_unused=True,
        )

    def put_inputs(self, in_maps):
        sharding = self.jax.sharding.NamedSharding(self.mesh, self.pspec)
        return [
            self.jax.device_put(
                np.concatenate([np.asarray(m[name]) for m in in_maps], axis=0),
                sharding)
            for name in self.in_names
        ]

    def _zeros(self):
        sharding = self.jax.sharding.NamedSharding(self.mesh, self.pspec)
        return [
            self.jax.device_put(
                np.zeros((self.n_cores * s[0], *s[1:]), d), sharding)
            for (s, d) in self.zero_shapes
        ]

    def run(self, dev_in):
        outs = self.fn(*dev_in, *self._zeros())
        self.jax.block_until_ready(outs)
        return outs

    def results(self, outs):
        res = []
        for c in range(self.n_cores):
            d = {}
            for i, name in enumerate(self.out_names):
                shp = self.out_avals[i].shape
                d[name] = np.asarray(outs[i]).reshape(
                    self.n_cores, *shp)[c]
            res.append(d)
        return res


def kernel(X, ref_a, ref_b, backref, e_map, v_count, W, W_back, W_prop,
           b, b_prop, **_unused):
    X = np.asarray(X)
    in_maps, cfg = _host_prep(X, ref_a, ref_b, backref,
                              W, W_back, W_prop, b, b_prop)
    key = ("v5", cfg["N"], cfg["gs_tot"], hash(cfg["group_meta"]))
    if key not in _cache:
        nc = _build_program(cfg)
        _cache[key] = SpmdRunner(nc)
    runner = _cache[key]
    dev_in = runner.put_inputs(in_maps)
    outs = runner.run(dev_in)
    res = runner.results(outs)
    full = np.concatenate([res[c]["out"] for c in range(N_CORES)], axis=0)
    return full[:cfg["N"]].astype(np.float32)
